# revision 72
# baseline (speedup 1.0000x reference)
"""Trainium2 Bass kernel for the CodedNet shift-mask-reduce problem.

Math (from the reference):
    out[b, i, j] = sum_c x[b, i, j, c] * bk[(i - c) % 256, j, c]

Strategy (v2 — DVE/Pool multiply + PE-matmul channel reduction):
  - Host: fuse the rolls into a per-channel rolled mask M[i, c, j]; it is
    128-periodic in both i and j for this problem's tiled-2x2 mask (verified
    at run time). Pack four i-values with the 28 channels onto the partition
    axis: p = di*28 + c with i = half*128 + ii*4 + di, giving
    x tiles [112, 32(ii), 256(j)] f16 and one shared bit-packed mask slab
    [112, 32(ii), 16] u8 (1 bit/element; identical for both i-halves;
    DVE bit-extracts + Act converts to f16, broadcast over the two j-halves
    on-chip with a stride-0 AP).
  - Shard batch 32 -> 4 per NeuronCore across 8 cores (pure data parallel).
  - Device, per tile (b, half) x 8, all under a ~43.6 us DMA critical path
    (x f16 40.8 + mask bits 0.16 + out f16 1.5 at the modeled 360 GB/s):
      DVE+Pool: one 16-bit 2x-mode tensor_mul y = x * mask per ii-chunk
           (Pool takes the last 4 ii of full chunks; DVE ~30 us, Pool ~22).
      PE:  channel reduction as matmuls with a block-ones stationary
           A[112, 4] (A[di*28+c, di'] = di==di'): psum[di', n] =
           sum_c y[(di',c), n] over n = 512-column chunks (two ii rows).
           Column tile_position 32*G places the 4-row outputs at psum
           partitions {32G..32G+3}; 2-bank psum slots x2 (WAR deps are
           psum-tile-granular); warmup matmuls hold PE at full clock.
      Act: psum -> SBUF f16 copyback (all 128 partitions flat; scattered-
           partition APs silently misread), staged per drain window.
      DMA: 3 drain windows (tiles 0-5, 6, 7) x 4 column-group DMAs (DMA
           APs allow only one contiguous partition dim), spread across the
           Pool/SP/Act queues; finer x chunks + a dedicated psum slot for
           the last tile shorten the post-last-byte tail to ~6 us.
  - Gather: un-permute [G, d, t, bank, iq, j] -> [b, i, j] and concat.
"""

import os

import numpy as np
import ml_dtypes

B, P, C = 32, 256, 28
N_CORES = 8
B_PER_CORE = B // N_CORES  # 4
N_TILES = B_PER_CORE * 2  # (b, iblk) -> 8 tiles of 128 i-rows
DI = 4
PPART = DI * C  # 112 partitions: p = di*28 + c
II = 128 // DI  # 32 i-subblocks per tile
NCHUNK = 2  # ii-chunks per tile (DMA/mul pipelining)
CII = II // NCHUNK  # 16 ii per chunk
MM_N = 512  # moving columns per matmul (= one psum bank of f32)
PAIR = MM_N // P  # ii rows covered per matmul = 2

DTYPE = os.environ.get("KERNEL_DTYPE", "f16")
if DTYPE not in ("f16", "bf16", "f32"):
    DTYPE = "f16"

_CACHE = {}
LAST_RESULTS = None  # stash of BassKernelResults for profiling from test harness


def _build(dtype_str=None):
    dtype_str = dtype_str or DTYPE
    key = (dtype_str,)
    if key in _CACHE:
        return _CACHE[key]

    import concourse.mybir as mybir
    from concourse import bacc, tile

    dt = {
        "bf16": mybir.dt.bfloat16,
        "f16": mybir.dt.float16,
        "f32": mybir.dt.float32,
    }[dtype_str]
    f32 = mybir.dt.float32

    nc = bacc.Bacc(
        "TRN2", target_bir_lowering=False, debug=False, num_devices=N_CORES
    )

    f8 = mybir.dt.float8e4

    xt = nc.dram_tensor("xt", [N_TILES, PPART, II, P], dt, kind="ExternalInput")
    u8 = mybir.dt.uint8
    mk = nc.dram_tensor("mk", [PPART, II, 16], u8, kind="ExternalInput")
    aw = nc.dram_tensor("aw", [PPART, DI], dt, kind="ExternalInput")
    # drain-order layout [G, di, tile, bank, iq, j]; host un-permutes
    # (row i of a tile = 32*bank + 8*G + 4*iq + di)
    out = nc.dram_tensor(
        "out", [4, DI, N_TILES, 4, PAIR, P], dt, kind="ExternalOutput"
    )

    xt_ap, mk_ap, out_ap = xt.ap(), mk.ap(), out.ap()

    with tile.TileContext(nc) as tc:
        with (
            tc.tile_pool(name="w", bufs=1) as wpool,
            tc.tile_pool(name="mask", bufs=1) as mpool,
            tc.tile_pool(name="x", bufs=4) as xpool,
            tc.tile_pool(name="o", bufs=1) as opool,
            tc.tile_pool(name="ps", bufs=2, space="PSUM") as ppool,
        ):
            m_t = mpool.tile([PPART, II, 128], dt)
            mbits = mpool.tile([PPART, II, 16], u8, name="mbits")
            m8 = mpool.tile([PPART, II, 128], u8, name="m8")
            # mask ships as 1 bit/element (57 KB); DVE extracts bits
            # ((bits >> b) & 1, bitVec ops are DVE-only and cannot cast) and
            # Act converts u8 -> f16, one 8-row quarter at a time so the
            # first multiply is not gated on the whole mask
            # SWDGE path: skips the shared HWDGE device, so the bit slab
            # lands before the first x transfer without a bus bubble
            nc.gpsimd.dma_start(out=mbits[:], in_=mk_ap[:])

            def mask_quarter(q):
                qsl = slice(8 * q, 8 * q + 8)
                m8v = m8[:, qsl, :].rearrange("p a (k b) -> p a k b", b=8)
                for b in range(8):
                    nc.vector.tensor_scalar(
                        out=m8v[:, :, :, b], in0=mbits[:, qsl, :],
                        scalar1=b, scalar2=1,
                        op0=mybir.AluOpType.logical_shift_right,
                        op1=mybir.AluOpType.bitwise_and)
                nc.scalar.copy(out=m_t[:, qsl, :], in_=m8[:, qsl, :])

            a_t = wpool.tile([PPART, DI], dt)

            # ii-chunk splits per tile: finer for the last tiles to shorten
            # the serial tail after the final x bytes land.  Copy/drain bank
            # groups follow the same idea (psum WAR deps are tile-granular,
            # so a copy of a psum tile blocks later matmuls into it; 2-bank
            # slots keep that harmless except for the last tile's half-slot
            # copies, which overlap the final small DMA chunks anyway).
            chunks = {t: [16, 16] for t in range(N_TILES)}
            chunks[0] = [8, 8, 8, 8]
            chunks[N_TILES - 2] = [12, 8, 6, 6]
            chunks[N_TILES - 1] = [8, 8, 6, 4, 4, 2]
            cgroups = {t: [2, 2] for t in range(N_TILES)}
            cgroups[N_TILES - 1] = [2, 1, 1]

            # PE p-state warmup: the tensor engine needs ~3us of continuous
            # work to reach full clock; a handful of zero matmuls starting
            # as soon as the stationary lands mean the real matmuls run at
            # 2.4 GHz from the start instead of crawling at 0.65 GHz.
            dummy = wpool.tile([PPART, MM_N], dt)
            nc.vector.memset(dummy[:], 0.0)

            # output staging tiles, one per drain window (tiles 0-5, 6, 7):
            # copies land here and each window drains as soon as its own
            # copies finish (a single shared tile would make every drain
            # conservatively wait on all later copies too)
            WINDOWS = [(0, N_TILES - 2), (N_TILES - 2, N_TILES - 1),
                       (N_TILES - 1, N_TILES)]
            sbw = [
                opool.tile(
                    [128, hi - lo, 4, MM_N], dt, name=f"sbw{w}", tag=f"sbw{w}"
                )
                for w, (lo, hi) in enumerate(WINDOWS)
            ]

            def sb_of(t):
                for w, (lo, hi) in enumerate(WINDOWS):
                    if lo <= t < hi:
                        return sbw[w], t - lo
                raise AssertionError(t)

            def drain(w, engines):
                # one DMA per column-group G: DMA APs support only a single
                # (contiguous) partition dim, so partitions 32G..32G+4 drain
                # separately (a scattered-partition AP reads garbage)
                t_lo, t_hi = WINDOWS[w]
                for G in range(4):
                    eng = engines[G % len(engines)]
                    eng.dma_start(
                        out=out_ap[G, :, t_lo:t_hi].rearrange(
                            "d t g q j -> d t (g q j)"
                        ),
                        in_=sbw[w][32 * G : 32 * G + DI],
                    )

            for t in range(N_TILES):
                x_t = xpool.tile([PPART, II, P], dt)
                # two 2-bank psum slots per x-tile; u -> bank u//4, grp u%4.
                # The last tile gives bank 3 its own slot (2nd psB ring buf)
                # so the bank-2 copyback can't WAR-block bank-3 matmuls.
                psA = ppool.tile([128, 2, MM_N], f32, name="psA")
                psB = ppool.tile([128, 2, MM_N], f32, name="psB")
                if t == N_TILES - 1:
                    psB2 = ppool.tile([128, 2, MM_N], f32, name="psB")
                    slot_of = {0: (psA, 0), 1: (psA, 1),
                               2: (psB, 0), 3: (psB2, 0)}
                else:
                    slot_of = {g: ((psA, psB)[g // 2], g % 2) for g in range(4)}
                if t == 0:
                    for _ in range(4):
                        nc.tensor.matmul(
                            out=psA[0:4, 0, :],
                            lhsT=dummy[:, 0:DI],
                            rhs=dummy[:],
                            start=True,
                            stop=True,
                            tile_position=(0, 0),
                        )
                done_u = 0  # ii-pairs whose matmul has been issued
                drained = 0  # banks copied back + drained
                groups = list(cgroups[t])
                ii0 = 0
                for ci, cii in enumerate(chunks[t]):
                    sl = slice(ii0, ii0 + cii)
                    if t == 0:
                        # keep the bit-extract one quarter ahead of the muls
                        if ci == 0:
                            mask_quarter(0)
                            mask_quarter(1)
                        elif ci < 3:
                            mask_quarter(ci + 1)
                    nc.sync.dma_start(out=x_t[:, sl, :], in_=xt_ap[t, :, sl, :])
                    if t == 0 and ci == 0:
                        # aw rides behind the first x transfer (a short DMA
                        # first in line leaves a DGE-latency bus bubble)
                        nc.sync.dma_start(out=a_t[:], in_=aw.ap())

                    # y = x * mask, mask broadcast over the two j-halves.
                    # DVE carries most of it; the otherwise-idle Pool engine
                    # takes the last 2 ii-rows of full chunks (~12%), pulling
                    # the compute tail in by ~4 us.
                    def mul(eng, lo, hi):
                        xv = x_t[:, lo:hi, :].rearrange(
                            "p a (h j) -> p a h j", h=2
                        )
                        mv = m_t[:, lo:hi, :].unsqueeze(2).broadcast_to(
                            [PPART, hi - lo, 2, 128]
                        )
                        eng.tensor_mul(out=xv, in0=xv, in1=mv)

                    if cii == 16:
                        mul(nc.vector, ii0, ii0 + 12)
                        mul(nc.gpsimd, ii0 + 12, ii0 + cii)
                    else:
                        mul(nc.vector, ii0, ii0 + cii)
                    # channel-reduce 512-column chunks on PE
                    for u in range(done_u, done_u + cii // PAIR):
                        g, G = u // 4, u % 4
                        st, col = slot_of[g]
                        nc.tensor.matmul(
                            out=st[32 * G : 32 * G + 4, col, :],
                            lhsT=a_t[:],
                            rhs=x_t[:, PAIR * u : PAIR * u + PAIR, :],
                            start=True,
                            stop=True,
                            tile_position=(0, 32 * G),
                        )
                    done_u += cii // PAIR
                    ii0 += cii
                    # psum -> SBUF f16 copyback on the Act engine (all 128
                    # partitions flat -- garbage lanes are free)
                    while groups and done_u // 4 >= drained + groups[0]:
                        ng = groups.pop(0)
                        sbt, trel = sb_of(t)
                        st, col = slot_of[drained]
                        nc.scalar.copy(
                            out=sbt[:, trel, drained : drained + ng, :],
                            in_=st[:, col : col + ng, :],
                        )
                        drained += ng
                if t == N_TILES - 3:
                    drain(0, (nc.gpsimd, nc.scalar))
                elif t == N_TILES - 2:
                    drain(1, (nc.scalar, nc.gpsimd))
                elif t == N_TILES - 1:
                    drain(2, (nc.gpsimd, nc.sync, nc.scalar, nc.sync))

    nc.compile()
    _CACHE[key] = nc
    return nc


def _prep_mask(bk, np_dt):
    """M[i, c, j] = bk[(i-c)%P, j, c]; if 128-periodic in i and j (always
    true for this problem's tiled-2x2, channel-repeated mask) return the
    packed slab Mp[di*28+c, ii, jm] = M[ii*4+di, c, jm], else None."""
    M = np.empty((P, C, P), dtype=np.float32)
    for c in range(C):
        M[:, c, :] = np.roll(bk[:, :, c], c, axis=0)
    per_i = np.array_equal(M[:128], M[128:])
    per_j = np.array_equal(M[:, :, :128], M[:, :, 128:])
    if not (per_i and per_j):
        return None
    M128 = M[:128, :, :128]  # [i=128, c, jm=128]
    Mp = M128.reshape(II, DI, C, 128).transpose(1, 2, 0, 3).reshape(PPART, II, 128)
    # shipped as 1 bit/element (LSB-first over jm), expanded on-chip
    bits = np.packbits(
        Mp.astype(np.uint8).reshape(PPART, II, 16, 8), axis=-1,
        bitorder="little",
    ).reshape(PPART, II, 16)
    return np.ascontiguousarray(bits)


def kernel(x: np.ndarray, bk: np.ndarray) -> np.ndarray:
    global LAST_RESULTS
    from concourse.bass_utils import run_bass_kernel_spmd

    x = np.asarray(x, dtype=np.float32)
    bk = np.asarray(bk, dtype=np.float32)
    np_dt = {"bf16": ml_dtypes.bfloat16, "f16": np.float16, "f32": np.float32}[
        DTYPE
    ]

    mk = _prep_mask(bk, np_dt)
    if mk is None:
        # generic fallback: no periodicity -> plain numpy
        return _kernel_generic(x, bk)

    # x [B, i, j, c] -> [B, half, di, c, ii, j] -> [cores, tiles, 112, 32, 256]
    xc = x.astype(np_dt)
    xt = np.ascontiguousarray(
        xc.reshape(B, 2, II, DI, P, C).transpose(0, 1, 3, 5, 2, 4)
    ).reshape(N_CORES, N_TILES, PPART, II, P)

    aw = np.zeros((PPART, DI), dtype=np_dt)
    for di in range(DI):
        aw[di * C : (di + 1) * C, di] = 1

    nc = _build()
    in_maps = [{"xt": xt[k], "mk": mk, "aw": aw} for k in range(N_CORES)]
    res = run_bass_kernel_spmd(nc, in_maps, core_ids=list(range(N_CORES)))
    LAST_RESULTS = res

    outs = []
    for k in range(N_CORES):
        o = res.results[k]["out"].reshape(4, DI, N_TILES, 4, PAIR, P)
        # [G, d, t, g, q, j] -> [t, g, G, q, d, j]; i = 32g + 8G + 4q + d
        o = o.transpose(2, 3, 0, 4, 1, 5).reshape(B_PER_CORE, P, P)
        outs.append(o.astype(np.float32))
    return np.concatenate(outs, axis=0)


def _kernel_generic(x: np.ndarray, bk: np.ndarray) -> np.ndarray:
    """Safety net for a non-periodic mask: plain numpy (never taken for the
    real problem inputs, whose mask is tiled 2x2 and channel-repeated)."""
    M = np.empty((P, C, P), dtype=np.float32)
    for c in range(C):
        M[:, c, :] = np.roll(bk[:, :, c], c, axis=0)
    # out[b,i,j] = sum_c x[b,i,j,c] * M[i,c,j]
    return np.einsum("bijc,icj->bij", x.astype(np.float32), M, optimize=True).astype(
        np.float32
    )
